# revision 1
# baseline (speedup 1.0000x reference)
"""Trainium2 Bass kernel for nn_Conv3DRecurrentInhibition.

The reference computes a 10-step linear fixed-point iteration
    state <- x + conv_C(state)           (15-tap conv along channels, zero pad)
which collapses to a single linear operator
    out[b, :, h, w] = T @ x[b, :, h, w],   T = sum_{k=0}^{max_steps} W^k
where W is the exact 256x256 banded matrix of the zero-padded conv
(cross-correlation orientation, matching lax.conv_general_dilated).
T is built on host (float64, from the 15-tap w_rec input). The device
computes the residual form y = x + T'@x with T' = T - I: the fp32r
matmul's rounding error then scales with the small T' products while x
passes through in exact fp32 via the DVE add.

Sharding: pure data parallel on batch — 32 samples over 8 cores, 4 each.
"""

import numpy as np

N_CORES = 8
B_FULL = 32
B_CORE = B_FULL // N_CORES  # 4
C = 256
HW = 56 * 56  # 3136
NTILE = 392  # 3136 = 8 * 392; >=256 keeps float32r matmul at full rate
TILES_PER_CHUNK = 2  # 784-col chunks: load/compute/store pipeline granularity
CHUNK = NTILE * TILES_PER_CHUNK
N_CHUNK = HW // CHUNK

_NC_CACHE = {}


def build_nc(reps: int = 1):
    """Build + compile the per-core Bass program.

    Per core: x [4, 256, 3136] f32, tT [128, 2, 256] f32 (T^T tiled so that
    tT[kp, kc, m] = T[m, kc*128 + kp]), y [4, 256, 3136] f32.
    reps>1 repeats the whole workload (for steady-state timing harnesses).
    """
    if reps in _NC_CACHE:
        return _NC_CACHE[reps]

    import concourse.bacc as bacc
    import concourse.mybir as mybir
    from concourse import tile

    f32 = mybir.dt.float32
    f32r = mybir.dt.float32r

    nc = bacc.Bacc("TRN2", target_bir_lowering=False, debug=False,
                   num_devices=N_CORES)
    # inputs feed the PE as fp32r (same 4-byte layout as fp32; full-rate
    # matmul at N>=256) — the BIR verifier requires the producing DMA to
    # already be typed fp32r
    x = nc.dram_tensor("x", [B_CORE, C, HW], f32r, kind="ExternalInput")
    tT = nc.dram_tensor("tT", [128, 2, C], f32r, kind="ExternalInput")
    y = nc.dram_tensor("y", [B_CORE, C, HW], f32, kind="ExternalOutput")

    with tile.TileContext(nc) as tc:
        with (
            tc.tile_pool(name="w", bufs=1) as wpool,
            tc.tile_pool(name="xin", bufs=8) as xpool,
            tc.tile_pool(name="out", bufs=8) as opool,
            tc.tile_pool(name="ps", bufs=8, space="PSUM") as pspool,
        ):
            wt = wpool.tile([128, 2, C], f32r)
            nc.gpsimd.dma_start(wt[:], tT[:])  # SWDGE: keep sync ring free for x loads

            for _ in range(reps):
                for b in range(B_CORE):
                    for c in range(N_CHUNK):
                        cs = slice(c * CHUNK, (c + 1) * CHUNK)
                        xa = xpool.tile([128, CHUNK], f32r, tag="xa")
                        xb = xpool.tile([128, CHUNK], f32r, tag="xb")
                        nc.sync.dma_start(xa[:], x[b, 0:128, cs])
                        nc.sync.dma_start(xb[:], x[b, 128:256, cs])
                        oa = opool.tile([128, CHUNK], f32, tag="oa")
                        ob = opool.tile([128, CHUNK], f32, tag="ob")
                        for n in range(TILES_PER_CHUNK):
                            sl = slice(n * NTILE, (n + 1) * NTILE)
                            for mc, ot, xh in ((0, oa, xa), (1, ob, xb)):
                                ps = pspool.tile([128, NTILE], f32, tag="ps")
                                nc.tensor.matmul(
                                    ps[:],
                                    wt[:, 0, mc * 128:(mc + 1) * 128],
                                    xa[:, sl],
                                    start=True, stop=False,
                                )
                                nc.tensor.matmul(
                                    ps[:],
                                    wt[:, 1, mc * 128:(mc + 1) * 128],
                                    xb[:, sl],
                                    start=False, stop=True,
                                )
                                # y = x + T'x (x re-added in exact fp32)
                                nc.vector.tensor_add(
                                    ot[:, sl], ps[:], xh[:, sl].bitcast(f32))
                        # stores on the ACT HWDGE ring so they overlap the
                        # sync-ring loads
                        nc.scalar.dma_start(y[b, 0:128, cs], oa[:])
                        nc.scalar.dma_start(y[b, 128:256, cs], ob[:])

    nc.compile()
    _NC_CACHE[reps] = nc
    return nc


def compose_T(w_rec: np.ndarray, max_steps: int, n_chan: int = C) -> np.ndarray:
    """T = sum_{k=0}^{max_steps} W^k for the zero-padded channel conv.

    lax.conv is cross-correlation: out_c = sum_dd w[dd] * y[c + dd - pad],
    so W[i, j] = w[j - i + pad].
    """
    w = np.asarray(w_rec, dtype=np.float64).reshape(-1)
    scope = w.shape[0]
    pad = scope // 2
    W = np.zeros((n_chan, n_chan), dtype=np.float64)
    for dd in range(scope):
        off = dd - pad
        d = np.diagonal(W, offset=off)
        d.setflags(write=True)
        d[:] = w[dd]
    eye = np.eye(n_chan, dtype=np.float64)
    acc = eye.copy()
    for _ in range(int(max_steps)):
        acc = eye + W @ acc
    return acc.astype(np.float32)


def make_in_maps(activations: np.ndarray, w_rec: np.ndarray, max_steps) -> list:
    acts = np.ascontiguousarray(np.asarray(activations, dtype=np.float32))
    assert acts.shape == (B_FULL, C, 56, 56), acts.shape
    T = compose_T(w_rec, int(np.asarray(max_steps)))
    Tp = T - np.eye(C, dtype=np.float32)  # residual operator T' = T - I
    # lhsT layout: tT[kp, kc, m] = T'^T[kc*128 + kp, m] = T'[m, kc*128 + kp]
    tTr = np.ascontiguousarray(Tp.T.reshape(2, 128, C).transpose(1, 0, 2))
    shards = acts.reshape(N_CORES, B_CORE, C, HW)
    return [{"x": shards[i], "tT": tTr} for i in range(N_CORES)]


def kernel(**inputs) -> np.ndarray:
    from concourse.bass_utils import run_bass_kernel_spmd

    in_maps = make_in_maps(inputs["activations"], inputs["w_rec"],
                           inputs["max_steps"])
    nc = build_nc(reps=1)
    res = run_bass_kernel_spmd(nc, in_maps, list(range(N_CORES)))
    out = np.stack([np.asarray(res.results[i]["y"]) for i in range(N_CORES)])
    return out.reshape(B_FULL, C, 56, 56).astype(np.float32, copy=False)



# revision 2
# speedup vs baseline: 1.8612x; 1.8612x over previous
"""Trainium2 Bass kernel for nn_Conv3DRecurrentInhibition.

The reference computes a 10-step linear fixed-point iteration
    state <- x + conv_C(state)           (15-tap conv along channels, zero pad)
which collapses to a single linear operator
    out[b, :, h, w] = T @ x[b, :, h, w],   T = sum_{k=0}^{max_steps} W^k
where W is the exact 256x256 banded matrix of the zero-padded conv
(cross-correlation orientation, matching lax.conv_general_dilated).
T is built on host (float64, from the 15-tap w_rec input).

The kernel is HBM-bandwidth bound (target_regime=ridge): per core it
must read 4x256x3136 activations and write the same volume back. All
device I/O is bf16 (rel-err budget 2e-2; bf16 costs ~3e-3), halving
HBM traffic vs fp32. The matmul runs bf16 x bf16 -> fp32 PSUM at full
PE rate; the PSUM->SBUF downcast copies alternate between the vector
and scalar engines so neither becomes the bottleneck.

Sharding: pure data parallel on batch — 32 samples over 8 cores, 4 each.
"""

import numpy as np

N_CORES = 8
B_FULL = 32
B_CORE = B_FULL // N_CORES  # 4
C = 256
HW = 56 * 56  # 3136
NTILE = 392  # psum tile: 392*4B = 1568B <= 2KB bank; 8 tiles cover 3136
TILES_PER_CHUNK = 4  # 1568-col chunks: 401KB per DMA at bf16
CHUNK = NTILE * TILES_PER_CHUNK
N_CHUNK = HW // CHUNK

_NC_CACHE = {}


def emit_body(nc, x, tT_wt, y, xpool, opool, pspool):
    """Emit one full per-core workload (all 4 samples)."""
    import concourse.mybir as mybir

    f32 = mybir.dt.float32
    bf16 = mybir.dt.bfloat16
    wt = tT_wt
    for b in range(B_CORE):
        for c in range(N_CHUNK):
            cs = slice(c * CHUNK, (c + 1) * CHUNK)
            xa = xpool.tile([128, CHUNK], bf16, tag="xa")
            xb = xpool.tile([128, CHUNK], bf16, tag="xb")
            nc.sync.dma_start(xa[:], x[b, 0:128, cs])
            nc.sync.dma_start(xb[:], x[b, 128:256, cs])
            oa = opool.tile([128, CHUNK], bf16, tag="oa")
            ob = opool.tile([128, CHUNK], bf16, tag="ob")
            for n in range(TILES_PER_CHUNK):
                sl = slice(n * NTILE, (n + 1) * NTILE)
                for mc, ot in ((0, oa), (1, ob)):
                    ps = pspool.tile([128, NTILE], f32, tag="ps")
                    nc.tensor.matmul(
                        ps[:], wt[:, 0, mc * 128:(mc + 1) * 128], xa[:, sl],
                        start=True, stop=False)
                    nc.tensor.matmul(
                        ps[:], wt[:, 1, mc * 128:(mc + 1) * 128], xb[:, sl],
                        start=False, stop=True)
                    # PSUM f32 -> SBUF bf16 downcast, split across engines
                    if (n + mc) % 2 == 0:
                        nc.vector.tensor_copy(ot[:, sl], ps[:])
                    else:
                        nc.scalar.copy(ot[:, sl], ps[:])
            # stores on the ACT HWDGE ring so they overlap the sync-ring loads
            nc.scalar.dma_start(y[b, 0:128, cs], oa[:])
            nc.scalar.dma_start(y[b, 128:256, cs], ob[:])


def build_nc(reps: int = 1):
    """Build + compile the per-core Bass program.

    Per core: x [4, 256, 3136] bf16, tT [128, 2, 256] bf16 (T^T tiled so
    that tT[kp, kc, m] = T[m, kc*128 + kp]), y [4, 256, 3136] bf16.
    reps>1 repeats the whole workload (for steady-state timing harnesses).
    """
    if reps in _NC_CACHE:
        return _NC_CACHE[reps]

    import concourse.bacc as bacc
    import concourse.mybir as mybir
    from concourse import tile

    bf16 = mybir.dt.bfloat16

    nc = bacc.Bacc("TRN2", target_bir_lowering=False, debug=False,
                   num_devices=N_CORES)
    x = nc.dram_tensor("x", [B_CORE, C, HW], bf16, kind="ExternalInput")
    tT = nc.dram_tensor("tT", [128, 2, C], bf16, kind="ExternalInput")
    y = nc.dram_tensor("y", [B_CORE, C, HW], bf16, kind="ExternalOutput")

    with tile.TileContext(nc) as tc:
        with (
            tc.tile_pool(name="w", bufs=1) as wpool,
            tc.tile_pool(name="xin", bufs=8) as xpool,
            tc.tile_pool(name="out", bufs=8) as opool,
            tc.tile_pool(name="ps", bufs=8, space="PSUM") as pspool,
        ):
            wt = wpool.tile([128, 2, C], bf16)
            nc.gpsimd.dma_start(wt[:], tT[:])  # SWDGE: keep sync ring free
            for _ in range(reps):
                emit_body(nc, x, wt, y, xpool, opool, pspool)

    nc.compile()
    _NC_CACHE[reps] = nc
    return nc


def compose_T(w_rec: np.ndarray, max_steps: int, n_chan: int = C) -> np.ndarray:
    """T = sum_{k=0}^{max_steps} W^k for the zero-padded channel conv.

    lax.conv is cross-correlation: out_c = sum_dd w[dd] * y[c + dd - pad],
    so W[i, j] = w[j - i + pad].
    """
    w = np.asarray(w_rec, dtype=np.float64).reshape(-1)
    scope = w.shape[0]
    pad = scope // 2
    W = np.zeros((n_chan, n_chan), dtype=np.float64)
    for dd in range(scope):
        off = dd - pad
        d = np.diagonal(W, offset=off)
        d.setflags(write=True)
        d[:] = w[dd]
    eye = np.eye(n_chan, dtype=np.float64)
    acc = eye.copy()
    for _ in range(int(max_steps)):
        acc = eye + W @ acc
    return acc.astype(np.float32)


def make_in_maps(activations: np.ndarray, w_rec: np.ndarray, max_steps) -> list:
    import ml_dtypes

    bf16 = ml_dtypes.bfloat16
    acts = np.asarray(activations, dtype=np.float32)
    assert acts.shape == (B_FULL, C, 56, 56), acts.shape
    T = compose_T(w_rec, int(np.asarray(max_steps)))
    # lhsT layout: tT[kp, kc, m] = T^T[kc*128 + kp, m] = T[m, kc*128 + kp]
    tTr = np.ascontiguousarray(
        T.T.reshape(2, 128, C).transpose(1, 0, 2)).astype(bf16)
    shards = np.ascontiguousarray(
        acts.astype(bf16).reshape(N_CORES, B_CORE, C, HW))
    return [{"x": shards[i], "tT": tTr} for i in range(N_CORES)]


def kernel(**inputs) -> np.ndarray:
    from concourse.bass_utils import run_bass_kernel_spmd

    in_maps = make_in_maps(inputs["activations"], inputs["w_rec"],
                           inputs["max_steps"])
    nc = build_nc(reps=1)
    res = run_bass_kernel_spmd(nc, in_maps, list(range(N_CORES)))
    out = np.stack([np.asarray(res.results[i]["y"]) for i in range(N_CORES)])
    return out.reshape(B_FULL, C, 56, 56).astype(np.float32)


# revision 7
# speedup vs baseline: 1.9369x; 1.0407x over previous
"""Trainium2 Bass kernel for nn_Conv3DRecurrentInhibition.

The reference computes a 10-step linear fixed-point iteration
    state <- x + conv_C(state)           (15-tap conv along channels, zero pad)
which collapses to a single linear operator
    out[b, :, h, w] = T @ x[b, :, h, w],   T = sum_{k=0}^{max_steps} W^k
where W is the exact 256x256 banded matrix of the zero-padded conv
(cross-correlation orientation, matching lax.conv_general_dilated).
T is built on host (float64, from the 15-tap w_rec input).

The kernel is HBM-bandwidth bound (target_regime=ridge): per core it
must read 4x256x3136 activations and write the same volume back, and
measured mixed read+write HBM rate is ~330 GB/s/core. Design:
  * all device I/O is bf16 (rel-err 6.7e-3 vs the 2e-2 budget),
    halving HBM traffic vs fp32;
  * host pre-swizzles x to [b, kp, kc, hw] (kc = channel/128) so each
    chunk moves with ONE dma_start whose per-partition runs are long
    and DRAM-contiguous; y is produced in the same layout and
    unswizzled on host after the gather;
  * loads ride the sync-engine HWDGE ring (the sync engine issues
    nothing else, so load doorbells never queue behind compute);
    stores ride the ACT ring; the 15-tap operator tile rides SWDGE;
  * PSUM->SBUF bf16 downcast copies alternate vector/scalar engines;
  * the first sample's first chunk and last sample's last chunk are
    small (2 of 8 tiles) to shrink pipeline ramp/drain, which the
    per-rep timing (and any single-shot profile) pays in full.

Sharding: pure data parallel on batch — 32 samples over 8 cores, 4 each.
"""

import numpy as np

N_CORES = 8
B_FULL = 32
B_CORE = B_FULL // N_CORES  # 4
C = 256
HW = 56 * 56  # 3136
NTILE = 392  # psum tile: 392*4B = 1568B <= 2KB bank; 8 tiles cover 3136
NT = HW // NTILE  # 8 tiles per sample row
# per-sample chunk schedule (t0, ntiles): taper the global first/last chunk
CHUNKS = [
    [(0, 2), (2, 6)],
    [(0, 8)],
    [(0, 8)],
    [(0, 6), (6, 2)],
]

# engines that issue chunk loads, round-robin by chunk index. Sync only:
# it is compute-free so its doorbells never queue behind compute, and
# HWDGE beats SWDGE (gpsimd) by ~6us on these 256-descriptor loads.
LOAD_ENGINES = ("sync",)

_NC_CACHE = {}


def declare_tensors(nc):
    import concourse.mybir as mybir

    bf16 = mybir.dt.bfloat16
    x = nc.dram_tensor("x", [B_CORE, 128, 2, HW], bf16, kind="ExternalInput")
    tT = nc.dram_tensor("tT", [128, 2, C], bf16, kind="ExternalInput")
    y = nc.dram_tensor("y", [B_CORE, 128, 2, HW], bf16, kind="ExternalOutput")
    return x, tT, y


def emit_body(nc, x, wt, y, xpool, opool, pspool):
    """Emit one full per-core workload (all 4 samples)."""
    import concourse.mybir as mybir

    f32 = mybir.dt.float32
    bf16 = mybir.dt.bfloat16
    ci = 0
    for b in range(B_CORE):
        for (t0, ct) in CHUNKS[b]:
            cs = slice(t0 * NTILE, (t0 + ct) * NTILE)
            ch = ct * NTILE
            xab = xpool.tile([128, 2, ch], bf16, tag=f"x{ct}")
            leng = getattr(nc, LOAD_ENGINES[ci % len(LOAD_ENGINES)])
            leng.dma_start(xab[:], x[b, :, :, cs])
            ci += 1
            oab = opool.tile([128, 2, ch], bf16, tag=f"o{ct}")
            for n in range(ct):
                sl = slice(n * NTILE, (n + 1) * NTILE)
                for mc in (0, 1):
                    ps = pspool.tile([128, NTILE], f32, tag="ps")
                    nc.tensor.matmul(
                        ps[:], wt[:, 0, mc * 128:(mc + 1) * 128], xab[:, 0, sl],
                        start=True, stop=False)
                    nc.tensor.matmul(
                        ps[:], wt[:, 1, mc * 128:(mc + 1) * 128], xab[:, 1, sl],
                        start=False, stop=True)
                    # PSUM f32 -> SBUF bf16 downcast, split across engines
                    if (n + mc) % 2 == 0:
                        nc.vector.tensor_copy(oab[:, mc, sl], ps[:])
                    else:
                        nc.scalar.copy(oab[:, mc, sl], ps[:])
            nc.scalar.dma_start(y[b, :, :, cs], oab[:])


def build_nc(reps: int = 1):
    """Build + compile the per-core Bass program.

    Per core: x [4, 128, 2, 3136] bf16 (x[b, kp, kc, hw] = act[b,
    kc*128 + kp, hw]), tT [128, 2, 256] bf16 (tT[kp, kc, m] =
    T[m, kc*128 + kp]), y like x. reps>1 repeats the whole workload
    (for steady-state timing harnesses).
    """
    if reps in _NC_CACHE:
        return _NC_CACHE[reps]

    import concourse.bacc as bacc
    import concourse.mybir as mybir
    from concourse import tile

    bf16 = mybir.dt.bfloat16

    nc = bacc.Bacc("TRN2", target_bir_lowering=False, debug=False,
                   num_devices=N_CORES)
    x, tT, y = declare_tensors(nc)

    with tile.TileContext(nc) as tc:
        with (
            tc.tile_pool(name="w", bufs=1) as wpool,
            tc.tile_pool(name="xin", bufs=3) as xpool,
            tc.tile_pool(name="out", bufs=3) as opool,
            tc.tile_pool(name="ps", bufs=8, space="PSUM") as pspool,
        ):
            wt = wpool.tile([128, 2, C], bf16)
            nc.gpsimd.dma_start(wt[:], tT[:])  # SWDGE: keep sync ring free
            for _ in range(reps):
                emit_body(nc, x, wt, y, xpool, opool, pspool)

    nc.compile()
    _NC_CACHE[reps] = nc
    return nc


def compose_T(w_rec: np.ndarray, max_steps: int, n_chan: int = C) -> np.ndarray:
    """T = sum_{k=0}^{max_steps} W^k for the zero-padded channel conv.

    lax.conv is cross-correlation: out_c = sum_dd w[dd] * y[c + dd - pad],
    so W[i, j] = w[j - i + pad].
    """
    w = np.asarray(w_rec, dtype=np.float64).reshape(-1)
    scope = w.shape[0]
    pad = scope // 2
    W = np.zeros((n_chan, n_chan), dtype=np.float64)
    for dd in range(scope):
        off = dd - pad
        d = np.diagonal(W, offset=off)
        d.setflags(write=True)
        d[:] = w[dd]
    eye = np.eye(n_chan, dtype=np.float64)
    acc = eye.copy()
    for _ in range(int(max_steps)):
        acc = eye + W @ acc
    return acc.astype(np.float32)


def make_in_maps(activations: np.ndarray, w_rec: np.ndarray, max_steps) -> list:
    import ml_dtypes

    bf16 = ml_dtypes.bfloat16
    acts = np.asarray(activations, dtype=np.float32)
    assert acts.shape == (B_FULL, C, 56, 56), acts.shape
    T = compose_T(w_rec, int(np.asarray(max_steps)))
    # lhsT layout: tT[kp, kc, m] = T^T[kc*128 + kp, m] = T[m, kc*128 + kp]
    tTr = np.ascontiguousarray(
        T.T.reshape(2, 128, C).transpose(1, 0, 2)).astype(bf16)
    # x[core, b, kp, kc, hw] = act[core, b, kc*128 + kp, hw]
    xs = acts.reshape(N_CORES, B_CORE, 2, 128, HW)
    xs = np.ascontiguousarray(xs.transpose(0, 1, 3, 2, 4)).astype(bf16)
    return [{"x": xs[i], "tT": tTr} for i in range(N_CORES)]


def unswizzle(y_global: np.ndarray) -> np.ndarray:
    """(32, 128, 2, HW) device layout -> (32, 256, 56, 56) fp32."""
    y = np.asarray(y_global).astype(np.float32).reshape(B_FULL, 128, 2, HW)
    return y.transpose(0, 2, 1, 3).reshape(B_FULL, C, 56, 56)


def kernel(**inputs) -> np.ndarray:
    from concourse.bass_utils import run_bass_kernel_spmd

    in_maps = make_in_maps(inputs["activations"], inputs["w_rec"],
                           inputs["max_steps"])
    nc = build_nc(reps=1)
    res = run_bass_kernel_spmd(nc, in_maps, list(range(N_CORES)))
    out = np.stack([np.asarray(res.results[i]["y"]) for i in range(N_CORES)])
    return unswizzle(out)


# revision 16
# speedup vs baseline: 2.0324x; 1.0493x over previous
"""Trainium2 Bass kernel for nn_Conv3DRecurrentInhibition.

The reference computes a 10-step linear fixed-point iteration
    state <- x + conv_C(state)           (15-tap conv along channels, zero pad)
which collapses to a single linear operator
    out[b, :, h, w] = T @ x[b, :, h, w],   T = sum_{k=0}^{max_steps} W^k
where W is the exact 256x256 banded matrix of the zero-padded conv
(cross-correlation orientation, matching lax.conv_general_dilated).
T is built on host (float64, from the 15-tap w_rec input).

The kernel is HBM-bandwidth bound (target_regime=ridge): per core it
must read 4x256x3136 activations and write the same volume back, and
measured mixed read+write HBM rate is ~330 GB/s/core. Design:
  * all device I/O is bf16 (rel-err 6.7e-3 vs the 2e-2 budget),
    halving HBM traffic vs fp32;
  * host pre-swizzles x to [b, kp, kc, hw] (kc = channel/128) so each
    chunk moves with ONE dma_start whose per-partition runs are long
    and DRAM-contiguous; y is produced in the same layout and
    unswizzled on host after the gather;
  * loads ride the sync-engine HWDGE ring (the sync engine issues
    nothing else, so load doorbells never queue behind compute);
    stores ride the ACT ring; the 15-tap operator tile rides SWDGE;
  * PSUM->SBUF bf16 downcast copies alternate vector/scalar engines;
  * the first sample's first chunk and last sample's last chunk are
    small (2 of 8 tiles) to shrink pipeline ramp/drain, which the
    per-rep timing (and any single-shot profile) pays in full.

Sharding: pure data parallel on batch — 32 samples over 8 cores, 4 each.
"""

import numpy as np

N_CORES = 8
B_FULL = 32
B_CORE = B_FULL // N_CORES  # 4
C = 256
HW = 56 * 56  # 3136
NTILE = 392  # psum tile: 392*4B = 1568B <= 2KB bank; 8 tiles cover 3136
NT = HW // NTILE  # 8 tiles per sample row
# per-sample chunk schedule (t0, ntiles): taper the global first/last chunk
CHUNKS = [
    [(0, 2), (2, 6)],
    [(0, 8)],
    [(0, 8)],
    [(0, 6), (6, 2)],
]

# engines that issue chunk loads, round-robin by chunk index. Sync only:
# it is compute-free so its doorbells never queue behind compute, and
# HWDGE beats SWDGE (gpsimd) by ~6us on these 256-descriptor loads.
# kc-split loads/stores (two 802KB transfers per middle chunk instead of
# one 1.6MB) measure ~1us faster and are far less sensitive to ambient
# HBM congestion than combined transfers: finer transfer granularity
# interleaves the read and write streams better (1.6MB loads + 1.6MB
# stores measured 303 GB/s vs ~330 GB/s at 802KB).
LOAD_ENGINES = ("sync",)
LOAD_SPLIT = "kc"    # False | "kc": two kc-half loads per chunk
SPLIT_STORE = "kc"   # False | "col": two half-width stores | "kc": two kc-half stores

_NC_CACHE = {}


def declare_tensors(nc):
    import concourse.mybir as mybir

    bf16 = mybir.dt.bfloat16
    x = nc.dram_tensor("x", [B_CORE, 128, 2, HW], bf16, kind="ExternalInput")
    tT = nc.dram_tensor("tT", [128, 2, C], bf16, kind="ExternalInput")
    y = nc.dram_tensor("y", [B_CORE, 128, 2, HW], bf16, kind="ExternalOutput")
    return x, tT, y


def emit_body(nc, x, wt, y, xpool, opool, pspool):
    """Emit one full per-core workload (all 4 samples)."""
    import concourse.mybir as mybir

    f32 = mybir.dt.float32
    bf16 = mybir.dt.bfloat16
    ci = 0
    for b in range(B_CORE):
        for (t0, ct) in CHUNKS[b]:
            cs = slice(t0 * NTILE, (t0 + ct) * NTILE)
            ch = ct * NTILE
            xab = xpool.tile([128, 2, ch], bf16, tag=f"x{ct}")
            leng = getattr(nc, LOAD_ENGINES[ci % len(LOAD_ENGINES)])
            if LOAD_SPLIT == "kc":
                leng.dma_start(xab[:, 0, :], x[b, :, 0, cs])
                leng.dma_start(xab[:, 1, :], x[b, :, 1, cs])
            else:
                leng.dma_start(xab[:], x[b, :, :, cs])
            ci += 1
            # split wide chunks into two stores so the store stream starts
            # half a chunk earlier and R/W interleaves at finer grain
            halves = ((0, ct),) if ct <= 4 or SPLIT_STORE not in ("col", "kc2") \
                else ((0, ct // 2), (ct // 2, ct - ct // 2))
            for (h0, hct) in halves:
                hs = slice((t0 + h0) * NTILE, (t0 + h0 + hct) * NTILE)
                oab = opool.tile([128, 2, hct * NTILE], bf16, tag=f"o{hct}")
                for n in range(hct):
                    sl = slice((h0 + n) * NTILE, (h0 + n + 1) * NTILE)
                    osl = slice(n * NTILE, (n + 1) * NTILE)
                    for mc in (0, 1):
                        ps = pspool.tile([128, NTILE], f32, tag="ps")
                        nc.tensor.matmul(
                            ps[:], wt[:, 0, mc * 128:(mc + 1) * 128],
                            xab[:, 0, sl], start=True, stop=False)
                        nc.tensor.matmul(
                            ps[:], wt[:, 1, mc * 128:(mc + 1) * 128],
                            xab[:, 1, sl], start=False, stop=True)
                        # PSUM f32 -> SBUF bf16 downcast, split across engines
                        if (n + mc) % 2 == 0:
                            nc.vector.tensor_copy(oab[:, mc, osl], ps[:])
                        else:
                            nc.scalar.copy(oab[:, mc, osl], ps[:])
                if SPLIT_STORE in ("kc", "kc2"):
                    nc.scalar.dma_start(y[b, :, 0, hs], oab[:, 0, :])
                    nc.scalar.dma_start(y[b, :, 1, hs], oab[:, 1, :])
                else:
                    nc.scalar.dma_start(y[b, :, :, hs], oab[:])


def build_nc(reps: int = 1):
    """Build + compile the per-core Bass program.

    Per core: x [4, 128, 2, 3136] bf16 (x[b, kp, kc, hw] = act[b,
    kc*128 + kp, hw]), tT [128, 2, 256] bf16 (tT[kp, kc, m] =
    T[m, kc*128 + kp]), y like x. reps>1 repeats the whole workload
    (for steady-state timing harnesses).
    """
    if reps in _NC_CACHE:
        return _NC_CACHE[reps]

    import concourse.bacc as bacc
    import concourse.mybir as mybir
    from concourse import tile

    bf16 = mybir.dt.bfloat16

    nc = bacc.Bacc("TRN2", target_bir_lowering=False, debug=False,
                   num_devices=N_CORES)
    x, tT, y = declare_tensors(nc)

    with tile.TileContext(nc) as tc:
        with (
            tc.tile_pool(name="w", bufs=1) as wpool,
            tc.tile_pool(name="xin", bufs=3) as xpool,
            tc.tile_pool(name="out", bufs=3) as opool,
            tc.tile_pool(name="ps", bufs=8, space="PSUM") as pspool,
        ):
            wt = wpool.tile([128, 2, C], bf16)
            nc.gpsimd.dma_start(wt[:], tT[:])  # SWDGE: keep sync ring free
            for _ in range(reps):
                emit_body(nc, x, wt, y, xpool, opool, pspool)

    nc.compile()
    _NC_CACHE[reps] = nc
    return nc


def compose_T(w_rec: np.ndarray, max_steps: int, n_chan: int = C) -> np.ndarray:
    """T = sum_{k=0}^{max_steps} W^k for the zero-padded channel conv.

    lax.conv is cross-correlation: out_c = sum_dd w[dd] * y[c + dd - pad],
    so W[i, j] = w[j - i + pad].
    """
    w = np.asarray(w_rec, dtype=np.float64).reshape(-1)
    scope = w.shape[0]
    pad = scope // 2
    W = np.zeros((n_chan, n_chan), dtype=np.float64)
    for dd in range(scope):
        off = dd - pad
        d = np.diagonal(W, offset=off)
        d.setflags(write=True)
        d[:] = w[dd]
    eye = np.eye(n_chan, dtype=np.float64)
    acc = eye.copy()
    for _ in range(int(max_steps)):
        acc = eye + W @ acc
    return acc.astype(np.float32)


def make_in_maps(activations: np.ndarray, w_rec: np.ndarray, max_steps) -> list:
    import ml_dtypes

    bf16 = ml_dtypes.bfloat16
    acts = np.asarray(activations, dtype=np.float32)
    assert acts.shape == (B_FULL, C, 56, 56), acts.shape
    T = compose_T(w_rec, int(np.asarray(max_steps)))
    # lhsT layout: tT[kp, kc, m] = T^T[kc*128 + kp, m] = T[m, kc*128 + kp]
    tTr = np.ascontiguousarray(
        T.T.reshape(2, 128, C).transpose(1, 0, 2)).astype(bf16)
    # x[core, b, kp, kc, hw] = act[core, b, kc*128 + kp, hw]
    xs = acts.reshape(N_CORES, B_CORE, 2, 128, HW)
    xs = np.ascontiguousarray(xs.transpose(0, 1, 3, 2, 4)).astype(bf16)
    return [{"x": xs[i], "tT": tTr} for i in range(N_CORES)]


def unswizzle(y_global: np.ndarray) -> np.ndarray:
    """(32, 128, 2, HW) device layout -> (32, 256, 56, 56) fp32."""
    y = np.asarray(y_global).astype(np.float32).reshape(B_FULL, 128, 2, HW)
    return y.transpose(0, 2, 1, 3).reshape(B_FULL, C, 56, 56)


def kernel(**inputs) -> np.ndarray:
    from concourse.bass_utils import run_bass_kernel_spmd

    in_maps = make_in_maps(inputs["activations"], inputs["w_rec"],
                           inputs["max_steps"])
    nc = build_nc(reps=1)
    res = run_bass_kernel_spmd(nc, in_maps, list(range(N_CORES)))
    out = np.stack([np.asarray(res.results[i]["y"]) for i in range(N_CORES)])
    return unswizzle(out)
